# revision 6
# baseline (speedup 1.0000x reference)
"""Multi-head attention (b=2, n=2048, d=1024, H=16 heads) on 8 TRN2 NeuronCores.

Sharding: core c = (b, g) with b = c // 4 (data parallel over batch) and
g = c % 4 (tensor parallel over head groups of 4 heads).  Each core computes
qkv projections for its 4 heads, full softmax attention for those heads, and
a partial output projection y_partial = A_heads @ w_out[g*256:(g+1)*256].
The host sums the 4 partials per batch and adds b_out.

Layout strategy (per core):
  - host passes xT = x[b].T  [1024, 2048]  (d on partitions when tiled)
  - qT, kT computed as [256, 2048] (head_dim*heads on partitions) via
    matmul(lhsT=w_slice, rhs=xT)  -> directly usable for scores matmuls
  - v computed as [2048, 256] (sequence on partitions) via
    matmul(lhsT=xT, rhs=wv), stored per-head with an appended ones column
    (v_aug [nk, 65]) so the PV matmul also accumulates the softmax denominator
  - scores computed TRANSPOSED: ST[nk, nq] = matmul(lhsT=kT, rhs=qT); softmax
    denominators come from the ones column; no row-max subtraction is needed
    because scores ~ N(0,1) (exp stays in a safe fp32 range)
  - exp on ScalarE directly from PSUM (fused *0.125 scale)
  - PV: outT[65, nq] += matmul(lhsT=v_aug, rhs=exp(ST))
  - normalize with reciprocal + K=1 ones matmul partition-broadcast
  - output projection from attn_outT [256, 2048]: y[n, d] tiles
All matmuls run in float32r (full PE rate at free dim >= 256, ~TF32 precision);
matmul-input tiles are declared float32r so producers round accordingly.
"""

import os
import sys

for _p in ("/opt/trn_rl_repo",):
    if _p not in sys.path and os.path.isdir(_p):
        sys.path.insert(0, _p)

import numpy as np

import concourse.bass as bass
import concourse.mybir as mybir
import concourse.tile as tile
from concourse import bacc

P = 128
D = 1024          # model dim
N = 2048          # sequence length
HD = 64           # head dim
GH = 4            # heads per core
DG = GH * HD      # 256 projected cols per core
KD = D // P       # 8 k-tiles over model dim
NT = N // P       # 16 tiles over sequence
QC = 512          # n_q chunk size
NQC = N // QC     # 4 chunks
SCALE = HD ** -0.5

F32 = mybir.dt.float32
F32R = mybir.dt.float32r

Exp = mybir.ActivationFunctionType.Exp


def build_nc():
    nc = bacc.Bacc("TRN2")

    xt = nc.declare_dram_parameter("xt", [D, N], F32, isOutput=False)
    wq = nc.declare_dram_parameter("wq", [D, DG], F32, isOutput=False)
    wk = nc.declare_dram_parameter("wk", [D, DG], F32, isOutput=False)
    wv = nc.declare_dram_parameter("wv", [D, DG], F32, isOutput=False)
    wo = nc.declare_dram_parameter("wo", [DG, D], F32, isOutput=False)
    y = nc.declare_dram_parameter("y", [N, D], F32, isOutput=True)

    def dram_r(t, pat):
        return t[:, :].rearrange(pat, p=P).bitcast(F32R)

    xt_r = dram_r(xt, "(o p) n -> p o n")    # [128, 8, 2048]
    wq_r = dram_r(wq, "(o p) n -> p o n")    # [128, 8, 256]
    wk_r = dram_r(wk, "(o p) n -> p o n")
    wv_r = dram_r(wv, "(o p) n -> p o n")
    wo_r = dram_r(wo, "(o p) n -> p o n")    # [128, 2, 1024]
    y_r = y[:, :].rearrange("(o p) n -> p o n", p=P)      # [128, 16, 1024]

    with tile.TileContext(nc) as tc, nc.allow_low_precision("fp32r matmul inputs"):
        with (
            tc.tile_pool(name="wpool", bufs=1) as wpool,
            tc.tile_pool(name="qkvpool", bufs=1) as qkvpool,
            tc.tile_pool(name="ps_a", bufs=2, space="PSUM") as ps_a,
        ):
            # --- load weights ---
            wk_sb = wpool.tile([P, KD, DG], F32R, tag="wk")
            nc.sync.dma_start(wk_sb[:], wk_r)
            wq_sb = wpool.tile([P, KD, DG], F32R, tag="wq")
            nc.sync.dma_start(wq_sb[:], wq_r)
            wv_sb = wpool.tile([P, KD, DG], F32R, tag="wv")
            nc.sync.dma_start(wv_sb[:], wv_r)
            wo_sb = wpool.tile([P, 2, D], F32R, tag="wo")
            nc.sync.dma_start(wo_sb[:], wo_r)
            ones_row = wpool.tile([1, HD], F32R, tag="ones")
            nc.scalar.copy(ones_row[:], nc.const_aps.tensor(1.0, (1, HD), F32))

            # --- persistent qkv tensors ---
            qt_sb = qkvpool.tile([P, 2, N], F32R, tag="qt")   # [256, 2048] qT
            kt_sb = qkvpool.tile([P, 2, N], F32R, tag="kt")   # [256, 2048] kT
            vg_sb = qkvpool.tile([P, NT, GH, 66], F32R, tag="vg")  # v + ones col
            nc.scalar.copy(
                vg_sb[:, :, :, HD:], nc.const_aps.tensor(1.0, (P, NT, GH, 2), F32)
            )

            # ---------------- qkv projection phase (uses xt) ----------------
            with tc.tile_pool(name="xpool", bufs=1) as xpool:
                xt_sb = xpool.tile([P, KD, N], F32R, tag="xt")
                for k in range(KD):
                    nc.sync.dma_start(xt_sb[:, k, :], xt_r[:, k, :])

                # v = x @ wv  -> [2048, 256], written per-head into vg_sb
                for t in range(NT):
                    ps = ps_a.tile([P, QC], F32, tag="a")
                    for k in range(KD):
                        nc.tensor.matmul(
                            ps[:, :DG],
                            xt_sb[:, k, t * P:(t + 1) * P],
                            wv_sb[:, k, :],
                            start=(k == 0),
                            stop=(k == KD - 1),
                        )
                    nc.scalar.copy(
                        vg_sb[:, t, :, 0:HD],
                        ps[:, :DG].rearrange("p (h e) -> p h e", h=GH),
                    )

                # kT then qT per m-tile (m = head pair), k-contiguous
                for m in range(2):
                    for which, w_sb, dst in (("k", wk_sb, kt_sb), ("q", wq_sb, qt_sb)):
                        for c in range(NQC):
                            ps = ps_a.tile([P, QC], F32, tag="a")
                            for k in range(KD):
                                nc.tensor.matmul(
                                    ps[:],
                                    w_sb[:, k, m * P:(m + 1) * P],
                                    xt_sb[:, k, c * QC:(c + 1) * QC],
                                    start=(k == 0),
                                    stop=(k == KD - 1),
                                )
                            nc.scalar.copy(dst[:, m, c * QC:(c + 1) * QC], ps[:])

            # ---------------- attention + out-projection ----------------
            with (
                tc.tile_pool(name="attnpool", bufs=1) as attnpool,
                tc.tile_pool(name="work", bufs=4) as work,
                tc.tile_pool(name="outp", bufs=2) as outp,
                tc.tile_pool(name="ps_st", bufs=4, space="PSUM") as ps_st,
                tc.tile_pool(name="ps_o", bufs=2, space="PSUM") as ps_o,
            ):
                at_sb = attnpool.tile([P, 2, N], F32R, tag="at")  # attn_outT [256, 2048]

                for c in range(NQC):
                    cs = slice(c * QC, (c + 1) * QC)
                    for pr in range(2):  # head pair (2pr, 2pr+1) on partition halves
                        o_ps = []
                        for half in range(2):
                            o_full = ps_o.tile([P, QC], F32, tag="o", name=f"o_{c}_{pr}_{half}")
                            o_ps.append(o_full[: HD + 1])
                        for t in range(NT):
                            ts_ = slice(t * P, (t + 1) * P)
                            e_sb = []
                            for half in range(2):
                                hs = slice(half * HD, (half + 1) * HD)
                                st = ps_st.tile([P, QC], F32, tag="st")
                                nc.tensor.matmul(
                                    st[:],
                                    kt_sb[hs, pr, ts_],
                                    qt_sb[hs, pr, cs],
                                    start=True,
                                    stop=True,
                                )
                                e = work.tile([P, QC], F32R, tag="exp")
                                nc.scalar.activation(e[:], st[:], Exp, scale=SCALE)
                                e_sb.append(e)
                            for half in range(2):
                                h = 2 * pr + half
                                nc.tensor.matmul(
                                    o_ps[half][:],
                                    vg_sb[:, t, h, 0:HD + 1],
                                    e_sb[half][:],
                                    start=(t == 0),
                                    stop=(t == NT - 1),
                                )
                        # normalize: A^T = outT[:64] * (1/outT[64]) broadcast
                        for half in range(2):
                            o = o_ps[half]
                            rc = work.tile([1, QC], F32R, tag="rc")
                            nc.vector.reciprocal(rc[:], o[HD:HD + 1, :])
                            rb_full = ps_st.tile([P, QC], F32, tag="st", name=f"rb_{c}_{pr}_{half}")
                            rb = rb_full[:HD]
                            nc.tensor.matmul(
                                rb[:], ones_row[:], rc[:],
                                start=True, stop=True,
                            )
                            rbs = work.tile([HD, QC], F32, tag="rbs")
                            nc.scalar.copy(rbs[:], rb[:])
                            if half == 0:
                                nc.vector.tensor_mul(
                                    at_sb[0:HD, pr, cs], o[0:HD, :], rbs[:]
                                )
                            else:
                                stg = work.tile([HD, QC], F32R, tag="stg")
                                nc.vector.tensor_mul(stg[:], o[0:HD, :], rbs[:])
                                nc.sync.dma_start(at_sb[HD:P, pr, cs], stg[:])

                    # out projection for this chunk's 4 row tiles
                    for mi in range(4):
                        m = 4 * c + mi
                        ysb = outp.tile([P, D], F32, tag="y")
                        for nn in range(2):
                            ps = ps_a.tile([P, QC], F32, tag="a")
                            for ks in range(2):
                                nc.tensor.matmul(
                                    ps[:],
                                    at_sb[:, ks, m * P:(m + 1) * P],
                                    wo_sb[:, ks, nn * QC:(nn + 1) * QC],
                                    start=(ks == 0),
                                    stop=(ks == 1),
                                )
                            if nn == 0:
                                nc.vector.tensor_copy(ysb[:, nn * QC:(nn + 1) * QC], ps[:])
                            else:
                                nc.scalar.copy(ysb[:, nn * QC:(nn + 1) * QC], ps[:])
                        nc.sync.dma_start(y_r[:, m, :], ysb[:])

    nc.finalize()
    return nc


_NC = None


def _get_nc():
    global _NC
    if _NC is None:
        _NC = build_nc()
    return _NC


def _in_maps(x, w_qkv, w_out):
    x = np.asarray(x, dtype=np.float32)
    w_qkv = np.asarray(w_qkv, dtype=np.float32)
    w_out = np.asarray(w_out, dtype=np.float32)
    xts = [np.ascontiguousarray(x[b].T) for b in range(2)]
    maps = []
    for c in range(8):
        b, g = c // 4, c % 4
        gs = slice(g * DG, (g + 1) * DG)
        maps.append({
            "xt": xts[b],
            "wq": np.ascontiguousarray(w_qkv[:, 0 * D:][:, gs]),
            "wk": np.ascontiguousarray(w_qkv[:, 1 * D:][:, gs]),
            "wv": np.ascontiguousarray(w_qkv[:, 2 * D:][:, gs]),
            "wo": np.ascontiguousarray(w_out[gs, :]),
        })
    return maps


LAST_RESULT = None


def kernel(x, w_qkv, w_out, b_out):
    from concourse.bass_utils import run_bass_kernel_spmd

    nc = _get_nc()
    maps = _in_maps(x, w_qkv, w_out)
    res = run_bass_kernel_spmd(nc, maps, list(range(8)))
    global LAST_RESULT
    LAST_RESULT = res
    out = np.zeros((2, N, D), dtype=np.float32)
    for c in range(8):
        out[c // 4] += res.results[c]["y"]
    out += np.asarray(b_out, dtype=np.float32)[None, None, :]
    return out


# revision 7
# speedup vs baseline: 1.5138x; 1.5138x over previous
"""Multi-head attention (b=2, n=2048, d=1024, H=16 heads) on 8 TRN2 NeuronCores.

Sharding: core c = (b, g) with b = c // 4 (data parallel over batch) and
g = c % 4 (tensor parallel over head groups of 4 heads).  Each core computes
qkv projections for its 4 heads, full softmax attention for those heads, and
a partial output projection y_partial = A_heads @ w_out[g*256:(g+1)*256].
The host sums the 4 partials per batch and adds b_out.

Layout strategy (per core):
  - host passes xT = x[b].T  [1024, 2048] in bf16 (d on partitions when tiled)
  - qT, kT computed as [256, 2048] (head_dim*heads on partitions) via
    matmul(lhsT=w_slice, rhs=xT)  -> directly usable for scores matmuls;
    head pairs (2m, 2m+1) sit on partition halves of m-tile m
  - v computed as [2048, 256] (sequence on partitions) via
    matmul(lhsT=xT, rhs=wv), stored per-head with an appended ones column
    (v_aug [nk, 65]) so the PV matmul also accumulates the softmax denominator
  - scores computed TRANSPOSED: ST[nk, nq] = matmul(lhsT=kT, rhs=qT); the two
    heads of a pair run concurrently in the PE array (row groups 0-63/64-127)
    and share one 2-bank PSUM tile so a single ACTIVATE exps 1024 elements
  - softmax needs no row-max subtraction (scores ~ N(0,1), exp <= ~3e3)
  - PV: outT[65, nq] += matmul(lhsT=v_aug, rhs=exp(ST))
  - normalize with DVE reciprocal + gpsimd partition_broadcast
  - output projection from attn_outT [256, 2048], interleaved per q-chunk
Matmuls run in bf16 (fp32 PSUM accumulation); measured end-to-end relative
error ~5e-3 vs the fp32 reference.
"""

import os
import sys

for _p in ("/opt/trn_rl_repo",):
    if _p not in sys.path and os.path.isdir(_p):
        sys.path.insert(0, _p)

import ml_dtypes
import numpy as np

import concourse.bass as bass
import concourse.mybir as mybir
import concourse.tile as tile
from concourse import bacc

P = 128
D = 1024          # model dim
N = 2048          # sequence length
HD = 64           # head dim
GH = 4            # heads per core
DG = GH * HD      # 256 projected cols per core
KD = D // P       # 8 k-tiles over model dim
NT = N // P       # 16 tiles over sequence
QC = 512          # n_q chunk size
NQC = N // QC     # 4 chunks
SCALE = HD ** -0.5

F32 = mybir.dt.float32
BF16 = mybir.dt.bfloat16

Exp = mybir.ActivationFunctionType.Exp


def build_nc():
    nc = bacc.Bacc("TRN2")

    xt = nc.declare_dram_parameter("xt", [D, N], BF16, isOutput=False)
    wq = nc.declare_dram_parameter("wq", [D, DG], BF16, isOutput=False)
    wk = nc.declare_dram_parameter("wk", [D, DG], BF16, isOutput=False)
    wv = nc.declare_dram_parameter("wv", [D, DG], BF16, isOutput=False)
    wo = nc.declare_dram_parameter("wo", [DG, D], BF16, isOutput=False)
    y = nc.declare_dram_parameter("y", [N, D], F32, isOutput=True)

    xt_r = xt[:, :].rearrange("(o p) n -> p o n", p=P)    # [128, 8, 2048]
    wq_r = wq[:, :].rearrange("(o p) n -> p o n", p=P)    # [128, 8, 256]
    wk_r = wk[:, :].rearrange("(o p) n -> p o n", p=P)
    wv_r = wv[:, :].rearrange("(o p) n -> p o n", p=P)
    wo_r = wo[:, :].rearrange("(o p) n -> p o n", p=P)    # [128, 2, 1024]
    y_r = y[:, :].rearrange("(o p) n -> p o n", p=P)      # [128, 16, 1024]

    with tile.TileContext(nc) as tc, nc.allow_low_precision("bf16 attention"):
        with (
            tc.tile_pool(name="wpool", bufs=1) as wpool,
            tc.tile_pool(name="qkvpool", bufs=1) as qkvpool,
            tc.tile_pool(name="ps_a", bufs=2, space="PSUM") as ps_a,
        ):
            # --- load weights ---
            wk_sb = wpool.tile([P, KD, DG], BF16, tag="wk")
            nc.sync.dma_start(wk_sb[:], wk_r)
            wq_sb = wpool.tile([P, KD, DG], BF16, tag="wq")
            nc.sync.dma_start(wq_sb[:], wq_r)
            wv_sb = wpool.tile([P, KD, DG], BF16, tag="wv")
            nc.sync.dma_start(wv_sb[:], wv_r)
            wo_sb = wpool.tile([P, 2, D], BF16, tag="wo")
            nc.sync.dma_start(wo_sb[:], wo_r)

            # --- persistent qkv tensors ---
            qt_sb = qkvpool.tile([P, 2, N], BF16, tag="qt")   # [256, 2048] qT
            kt_sb = qkvpool.tile([P, 2, N], BF16, tag="kt")   # [256, 2048] kT
            vg_sb = qkvpool.tile([P, NT, GH, 66], BF16, tag="vg")  # v + ones col
            nc.scalar.copy(
                vg_sb[:, :, :, HD:], nc.const_aps.tensor(1.0, (P, NT, GH, 2), F32)
            )

            # ---------------- qkv projection phase (uses xt) ----------------
            with tc.tile_pool(name="xpool", bufs=1) as xpool:
                xt_sb = xpool.tile([P, KD, N], BF16, tag="xt")
                for k in range(KD):
                    nc.sync.dma_start(xt_sb[:, k, :], xt_r[:, k, :])

                # v = x @ wv  -> [2048, 256], written per-head into vg_sb
                for t in range(NT):
                    ps = ps_a.tile([P, QC], F32, tag="a")
                    for k in range(KD):
                        nc.tensor.matmul(
                            ps[:, :DG],
                            xt_sb[:, k, t * P:(t + 1) * P],
                            wv_sb[:, k, :],
                            start=(k == 0),
                            stop=(k == KD - 1),
                        )
                    nc.scalar.copy(
                        vg_sb[:, t, :, 0:HD],
                        ps[:, :DG].rearrange("p (h e) -> p h e", h=GH),
                    )

                # kT then qT per m-tile (m = head pair), k-contiguous
                for m in range(2):
                    for which, w_sb, dst in (("k", wk_sb, kt_sb), ("q", wq_sb, qt_sb)):
                        for c in range(NQC):
                            ps = ps_a.tile([P, QC], F32, tag="a")
                            for k in range(KD):
                                nc.tensor.matmul(
                                    ps[:],
                                    w_sb[:, k, m * P:(m + 1) * P],
                                    xt_sb[:, k, c * QC:(c + 1) * QC],
                                    start=(k == 0),
                                    stop=(k == KD - 1),
                                )
                            nc.scalar.copy(dst[:, m, c * QC:(c + 1) * QC], ps[:])

            # ---------------- attention + out-projection ----------------
            with (
                tc.tile_pool(name="attnpool", bufs=1) as attnpool,
                tc.tile_pool(name="work", bufs=4) as work,
                tc.tile_pool(name="outp", bufs=2) as outp,
                tc.tile_pool(name="ps_st", bufs=2, space="PSUM") as ps_st,
                tc.tile_pool(name="ps_o", bufs=2, space="PSUM") as ps_o,
            ):
                at_sb = attnpool.tile([P, 2, N], BF16, tag="at")  # attn_outT [256, 2048]

                for c in range(NQC):
                    cs = slice(c * QC, (c + 1) * QC)
                    for pr in range(2):  # head pair (2pr, 2pr+1) on partition halves
                        o_ps = []
                        for half in range(2):
                            o_full = ps_o.tile([P, QC], F32, tag="o", name=f"o_{c}_{pr}_{half}")
                            o_ps.append(o_full[: HD + 1])
                        for t in range(NT):
                            ts_ = slice(t * P, (t + 1) * P)
                            # both heads' transposed scores in one 2-bank tile
                            st = ps_st.tile([P, 2, QC], F32, tag="st")
                            for half in range(2):
                                hs = slice(half * HD, (half + 1) * HD)
                                nc.tensor.matmul(
                                    st[:, half, :],
                                    kt_sb[hs, pr, ts_],
                                    qt_sb[hs, pr, cs],
                                    start=True,
                                    stop=True,
                                )
                            e = work.tile([P, 2, QC], BF16, tag="exp")
                            nc.scalar.activation(e[:], st[:], Exp, scale=SCALE)
                            for half in range(2):
                                h = 2 * pr + half
                                nc.tensor.matmul(
                                    o_ps[half][:],
                                    vg_sb[:, t, h, 0:HD + 1],
                                    e[:, half, :],
                                    start=(t == 0),
                                    stop=(t == NT - 1),
                                )
                        # normalize: A^T = outT[:64] * (1/outT[64]) broadcast
                        for half in range(2):
                            o = o_ps[half]
                            rc = work.tile([1, QC], F32, tag="rc")
                            nc.vector.reciprocal(rc[:], o[HD:HD + 1, :])
                            rbs = work.tile([HD, QC], F32, tag="rbs")
                            nc.gpsimd.partition_broadcast(rbs[:], rc[:])
                            if half == 0:
                                nc.vector.tensor_mul(
                                    at_sb[0:HD, pr, cs], o[0:HD, :], rbs[:]
                                )
                            else:
                                stg = work.tile([HD, QC], BF16, tag="stg")
                                nc.vector.tensor_mul(stg[:], o[0:HD, :], rbs[:])
                                nc.sync.dma_start(at_sb[HD:P, pr, cs], stg[:])

                    # out projection for this chunk's 4 row tiles
                    for mi in range(4):
                        m = 4 * c + mi
                        ysb = outp.tile([P, D], F32, tag="y")
                        for nn in range(2):
                            ps = ps_a.tile([P, QC], F32, tag="a")
                            for ks in range(2):
                                nc.tensor.matmul(
                                    ps[:],
                                    at_sb[:, ks, m * P:(m + 1) * P],
                                    wo_sb[:, ks, nn * QC:(nn + 1) * QC],
                                    start=(ks == 0),
                                    stop=(ks == 1),
                                )
                            nc.vector.tensor_copy(ysb[:, nn * QC:(nn + 1) * QC], ps[:])
                        nc.sync.dma_start(y_r[:, m, :], ysb[:])

    nc.finalize()
    return nc


_NC = None


def _get_nc():
    global _NC
    if _NC is None:
        _NC = build_nc()
    return _NC


def _in_maps(x, w_qkv, w_out):
    bf = ml_dtypes.bfloat16
    x = np.asarray(x, dtype=np.float32)
    w_qkv = np.asarray(w_qkv, dtype=np.float32)
    w_out = np.asarray(w_out, dtype=np.float32)
    xts = [np.ascontiguousarray(x[b].T).astype(bf) for b in range(2)]
    wq_g = [np.ascontiguousarray(w_qkv[:, 0 * D + g * DG:0 * D + (g + 1) * DG]).astype(bf) for g in range(4)]
    wk_g = [np.ascontiguousarray(w_qkv[:, 1 * D + g * DG:1 * D + (g + 1) * DG]).astype(bf) for g in range(4)]
    wv_g = [np.ascontiguousarray(w_qkv[:, 2 * D + g * DG:2 * D + (g + 1) * DG]).astype(bf) for g in range(4)]
    wo_g = [np.ascontiguousarray(w_out[g * DG:(g + 1) * DG, :]).astype(bf) for g in range(4)]
    maps = []
    for c in range(8):
        b, g = c // 4, c % 4
        maps.append({
            "xt": xts[b],
            "wq": wq_g[g],
            "wk": wk_g[g],
            "wv": wv_g[g],
            "wo": wo_g[g],
        })
    return maps


LAST_RESULT = None


def kernel(x, w_qkv, w_out, b_out):
    from concourse.bass_utils import run_bass_kernel_spmd

    nc = _get_nc()
    maps = _in_maps(x, w_qkv, w_out)
    res = run_bass_kernel_spmd(nc, maps, list(range(8)))
    global LAST_RESULT
    LAST_RESULT = res
    out = np.zeros((2, N, D), dtype=np.float32)
    for c in range(8):
        out[c // 4] += res.results[c]["y"]
    out += np.asarray(b_out, dtype=np.float32)[None, None, :]
    return out
